# revision 26
# baseline (speedup 1.0000x reference)
"""Trainium2 Bass kernel for the single-query-attention diffusion decoder.

Full-input contract: kernel(**inputs) -> np.ndarray [B, V].
Data-parallel over batch across 8 NeuronCores (16 rows each).

Math (reference restructured):
    cond  = silu(pe[t] @ Wt1.T + bt1) @ Wt2.T + bt2            [B, D]
    q~    = (query + cond) @ M1,  M1 = Wq.T @ Wk               [B, D]
    s[v]  = q~ . T[v] + x[v]   (softmax shift-invariant)
    w     = softmax(s)
    ws    = sum_v w[v] T[v] + cond                             [D]
    base  = ws @ M3 + r0                                       [J]
            M3 = Wv.T @ Wp.T @ Wd1[:, :D].T,  r0 = bp @ Wd1[:, :D].T + bd1
    p[v]  = sum_j relu(T[v] @ Bm + base)[j] * w2[j] + bd2 + w[v]
            Bm = Wd1[:, D:].T

v2 layout: the decoder H matmul runs in [j, v] orientation (lhsT = Bm
chunk stationary, te streams) so the per-j base fold is a free ACT-bias
in the relu, and the w2 reduction over j is a K=128 matvec.  Rows are
processed in groups of 4; the M=1 matvecs (scores, w2-reduce) for the 4
rows are column-packed into one PSUM bank at partitions 0/32/64/96 and
emitted back-to-back so they overlap in the PE array.  Shared banks are
initialized by a single full-partition K=4 "selector" matmul that also
folds the additive row term (x for scores, softmax weights for p), and
all per-row chains accumulate with start=False.  The base matvec is
batched (ws of 4 rows as N) and emitted chunk-wise between H bursts.
"""

import os
import sys

for _p in ("/opt/trn_rl_repo", "/opt/trn_rl_repo/concourse"):
    if os.path.isdir(_p) and _p not in sys.path:
        sys.path.append(_p)

import numpy as np
import ml_dtypes

import concourse.bass as bass
import concourse.tile as tile
from concourse import bacc, mybir
from concourse.bass_utils import run_bass_kernel_spmd

F32 = mybir.dt.float32
BF16 = mybir.dt.bfloat16
I32 = mybir.dt.int32
AF = mybir.ActivationFunctionType
ALU = mybir.AluOpType
BF_NP = ml_dtypes.bfloat16

NCORES = 8
B = 128
BSH = B // NCORES  # 16 batch rows per core
D = 512
V = 1024
J = 2 * D  # 1024 decoder hidden
DC = D // 128  # 4 d-chunks
JB = J // 128  # 8 j-chunks
GR = 2  # rows per group (column-packed)
NG = BSH // GR  # 4 groups
MAX_LEN = 5000
NO_SPRINKLE = bool(int(os.environ.get("K_NO_SPRINKLE", "0")))
NO_DELAY = bool(int(os.environ.get("K_NO_DELAY", "0")))
# 0=defer all next-group prep, 1=sprinkle loads, 2=+scores, 3=+softmax/ws (all)
SPR_LEVEL = int(os.environ.get("K_SPR_LEVEL", "5"))


def build_nc(bd2_val: float) -> bass.Bass:
    nc = bacc.Bacc()

    # ---- per-core inputs ----
    te_d = nc.declare_dram_parameter("te", [BSH, D, V], BF16, isOutput=False)
    x_d = nc.declare_dram_parameter("x", [BSH, V], BF16, isOutput=False)
    ts_d = nc.declare_dram_parameter("ts", [BSH, 1], I32, isOutput=False)
    qet_d = nc.declare_dram_parameter("qet", [D, BSH], BF16, isOutput=False)
    # ---- replicated (host-folded) weights ----
    pe_d = nc.declare_dram_parameter("pe", [MAX_LEN, D], F32, isOutput=False)
    wt1t_d = nc.declare_dram_parameter("wt1t", [D, D], BF16, isOutput=False)
    wt2t_d = nc.declare_dram_parameter("wt2t", [D, D], BF16, isOutput=False)
    bt1c_d = nc.declare_dram_parameter("bt1c", [128, DC], F32, isOutput=False)
    bt2c_d = nc.declare_dram_parameter("bt2c", [128, DC], F32, isOutput=False)
    m1_d = nc.declare_dram_parameter("m1", [D, D], BF16, isOutput=False)
    m3_d = nc.declare_dram_parameter("m3", [D, J], BF16, isOutput=False)
    bm_d = nc.declare_dram_parameter("bm", [D, J], BF16, isOutput=False)
    r0_d = nc.declare_dram_parameter("r0", [J], BF16, isOutput=False)
    w2c_d = nc.declare_dram_parameter("w2c", [128, JB], BF16, isOutput=False)
    sel4_d = nc.declare_dram_parameter("sel4", [128, 128], BF16, isOutput=False)
    p_d = nc.declare_dram_parameter("p", [BSH, V], F32, isOutput=True)

    with tile.TileContext(nc) as tc:
        with (
            tc.tile_pool(name="w", bufs=1) as wp,
            tc.tile_pool(name="te", bufs=12) as tep,
            tc.tile_pool(name="scr", bufs=1) as scrp,
            tc.tile_pool(name="rows", bufs=2) as rowp,
            tc.tile_pool(name="hr", bufs=8) as hrp,
            tc.tile_pool(name="tiny", bufs=6) as tinyp,
            tc.tile_pool(name="grp", bufs=2) as grpp,
            tc.tile_pool(name="ebsb", bufs=2) as ebsb,
            tc.tile_pool(name="dramp", bufs=2, space="DRAM") as dramp,
            tc.tile_pool(name="hp", bufs=4, space="PSUM") as hp,  # 4 banks
            tc.tile_pool(name="pp", bufs=1, space="PSUM") as pp,  # 2 banks
            tc.tile_pool(name="scp", bufs=2, space="PSUM") as scp,  # 2 banks
        ):
            # ================= weight / constant loads =================
            # setup-critical small tensors first so the cond/q~ chain and the
            # first group's scores aren't queued behind bulk weights
            ts_sb = wp.tile([BSH, 1], I32, tag="ts")
            nc.sync.dma_start(out=ts_sb, in_=ts_d[:])
            bt1c = wp.tile([128, DC], F32, tag="bt1c")
            nc.sync.dma_start(out=bt1c, in_=bt1c_d[:])
            bt2c = wp.tile([128, DC], F32, tag="bt2c")
            nc.sync.dma_start(out=bt2c, in_=bt2c_d[:])
            qet = wp.tile([128, DC, BSH], BF16, tag="qet")
            nc.sync.dma_start(out=qet, in_=qet_d[:].rearrange("(c p) b -> p c b", p=128))
            wt1t = wp.tile([128, DC, D], BF16, tag="wt1t")
            nc.sync.dma_start(out=wt1t, in_=wt1t_d[:].rearrange("(c p) z -> p c z", p=128))
            wt2t = wp.tile([128, DC, D], BF16, tag="wt2t")
            nc.sync.dma_start(out=wt2t, in_=wt2t_d[:].rearrange("(c p) z -> p c z", p=128))
            m1 = wp.tile([128, DC, D], BF16, tag="m1")
            nc.sync.dma_start(out=m1, in_=m1_d[:].rearrange("(c p) z -> p c z", p=128))
            m3 = wp.tile([128, DC, J], BF16, tag="m3")
            bm = wp.tile([128, DC, J], BF16, tag="bm")
            # r0 as a single row on partition 0 (K=1 lhsT for the base fold)
            r01 = wp.tile([1, J], BF16, tag="r01")
            nc.sync.dma_start(
                out=r01, in_=bass.AP(tensor=r0_d, offset=0, ap=[[J, 1], [1, J]])
            )
            # w2 chunked [p, jb] (j = jb*128 + p); lhsT columns for the reduce
            w2sb = wp.tile([128, JB], BF16, tag="w2sb")
            nc.sync.dma_start(out=w2sb, in_=w2c_d[:])
            ones_bf = wp.tile([1, 128], BF16, tag="ones_bf")
            nc.vector.memset(ones_bf, 1.0)
            # selector: sel4[32k, 32k] = 1, else 0 -- bank-init matmuls
            sel4 = wp.tile([128, 128], BF16, tag="sel4")
            nc.sync.dma_start(out=sel4, in_=sel4_d[:])
            id128 = wp.tile([128, 128], F32, tag="id128")
            from concourse.masks import make_identity

            make_identity(nc, id128)
            id_bf = wp.tile([BSH, BSH], BF16, tag="id_bf")
            nc.scalar.activation(out=id_bf, in_=id128[:BSH, :BSH], func=AF.Copy)
            # PE warmup so later transposes never owe a Pool wait
            warm_ps = hp.tile([2, 2], F32, tag="h")
            nc.tensor.transpose(warm_ps, id128[0:2, 0:2], id128[0:2, 0:2])
            zero1 = wp.tile([128, 1], F32, tag="zero1")
            nc.vector.memset(zero1, 0.0)
            zerosV = wp.tile([128, V], BF16, tag="zerosV")
            nc.vector.memset(zerosV, 0.0)

            # ================= setup: cond / q~ =================
            tpe = wp.tile([BSH, D], F32, tag="tpe")
            nc.gpsimd.indirect_dma_start(
                out=tpe[:],
                out_offset=None,
                in_=pe_d[:],
                in_offset=bass.IndirectOffsetOnAxis(ap=ts_sb[:, :1], axis=0),
            )
            tpe_bf = wp.tile([BSH, D], BF16, tag="tpe_bf")
            nc.scalar.activation(out=tpe_bf, in_=tpe, func=AF.Copy)
            tpeT = wp.tile([128, DC, BSH], BF16, tag="tpeT")
            for c in range(DC):
                ps = hp.tile([128, BSH], BF16, tag="h", name=f"tps{c}")
                nc.tensor.transpose(ps, tpe_bf[:, c * 128 : (c + 1) * 128], id_bf)
                nc.scalar.activation(out=tpeT[:, c, :], in_=ps, func=AF.Copy)
            # Z.T = Wt1 @ tpe.T (+bt1), silu
            s_sb = wp.tile([128, DC, BSH], BF16, tag="s_sb")
            for zt in range(DC):
                ps = hp.tile([128, BSH], F32, tag="h", name=f"zps{zt}")
                for c in range(DC):
                    nc.tensor.matmul(
                        ps, wt1t[:, c, zt * 128 : (zt + 1) * 128], tpeT[:, c, :],
                        start=(c == 0), stop=(c == DC - 1),
                    )
                # silu(z) = z * sigmoid(z); sim has no Silu
                zt_sb = wp.tile([128, DC, BSH], BF16, tag="zt_sb")
                nc.scalar.activation(
                    out=zt_sb[:, zt, :], in_=ps, func=AF.Identity,
                    bias=bt1c[:, zt : zt + 1], scale=1.0,
                )
                sg_sb = wp.tile([128, DC, BSH], BF16, tag="sg_sb")
                nc.scalar.activation(
                    out=sg_sb[:, zt, :], in_=ps, func=AF.Sigmoid,
                    bias=bt1c[:, zt : zt + 1], scale=1.0,
                )
                nc.vector.tensor_mul(
                    s_sb[:, zt, :], zt_sb[:, zt, :], sg_sb[:, zt, :]
                )
            # condT = Wt2 @ silu (+bt2)
            condT = wp.tile([128, DC, BSH], BF16, tag="condT")
            for ct in range(DC):
                ps = hp.tile([128, BSH], F32, tag="h", name=f"cps{ct}")
                for c in range(DC):
                    nc.tensor.matmul(
                        ps, wt2t[:, c, ct * 128 : (ct + 1) * 128], s_sb[:, c, :],
                        start=(c == 0), stop=(c == DC - 1),
                    )
                nc.scalar.activation(
                    out=condT[:, ct, :], in_=ps, func=AF.Identity,
                    bias=bt2c[:, ct : ct + 1], scale=1.0,
                )
            # qcT = qeT + condT ; q~T = M1.T @ qcT
            qcT = wp.tile([128, DC, BSH], BF16, tag="qcT")
            nc.vector.tensor_add(qcT[:], qet[:], condT[:])
            qtT = wp.tile([128, DC, BSH], BF16, tag="qtT")
            for mt in range(DC):
                ps = hp.tile([128, BSH], F32, tag="h", name=f"qps{mt}")
                for c in range(DC):
                    nc.tensor.matmul(
                        ps, m1[:, c, mt * 128 : (mt + 1) * 128], qcT[:, c, :],
                        start=(c == 0), stop=(c == DC - 1),
                    )
                nc.scalar.activation(out=qtT[:, mt, :], in_=ps, func=AF.Copy)

            # ================= per-row / per-group state =================
            st = [dict() for _ in range(BSH)]
            gst = [dict() for _ in range(NG)]

            def emit_x4_load(g):
                gs = gst[g]
                x4 = grpp.tile([128, V], BF16, tag="x4", name=f"x4_{g}")
                nc.vector.memset(x4, 0.0)
                for i in range(GR):
                    b = g * GR + i
                    nc.sync.dma_start(
                        out=x4[32 * i : 32 * i + 1, :], in_=x_d[b : b + 1, :]
                    )
                gs["x4"] = x4

            def emit_te_load(g, i):
                b = g * GR + i
                te_t = tep.tile([128, DC, V], BF16, tag="te", name=f"te{b}")
                for c in range(DC):
                    nc.sync.dma_start(
                        out=te_t[:, c, :], in_=te_d[b, c * 128 : (c + 1) * 128, :]
                    )
                st[b]["te"] = te_t

            def emit_scores(g):
                """sc[j32i, v] for 4 rows column-packed; x folded via bank init."""
                gs = gst[g]
                sc = []
                for h in range(2):
                    scps = scp.tile([128, 512], F32, tag="sc", name=f"sc{g}_{h}")
                    nc.tensor.matmul(
                        scps, sel4, gs["x4"][:, h * 512 : (h + 1) * 512],
                        start=True, stop=False, skip_group_check=True,
                    )
                    for c in range(DC):
                        for i in range(GR):
                            b = g * GR + i
                            nc.tensor.matmul(
                                scps[32 * i : 32 * i + 1, :],
                                qtT[:, c, b : b + 1],
                                st[b]["te"][:, c, h * 512 : (h + 1) * 512],
                                start=False, stop=(c == DC - 1),
                                skip_group_check=True,
                                tile_position=(0, 32 * i),
                            )
                    sc.append(scps)
                gs["sc"] = sc

            def emit_softmax(g, i):
                gs = gst[g]
                b = g * GR + i
                if "expn4" not in gs:
                    gs["expn4"] = grpp.tile([128, V], BF16, tag="expn4", name=f"en{g}")
                    nc.vector.memset(gs["expn4"], 0.0)
                exp_row = rowp.tile([1, V], F32, tag="exp", name=f"exp{b}")
                se = [
                    tinyp.tile([1, 1], F32, tag="t1", name=f"se{h}_{b}")
                    for h in range(2)
                ]
                for h in range(2):
                    nc.scalar.activation(
                        out=exp_row[:, h * 512 : (h + 1) * 512],
                        in_=gs["sc"][h][32 * i : 32 * i + 1, :],
                        func=AF.Exp, accum_out=se[h],
                    )
                sume = tinyp.tile([1, 1], F32, tag="t1", name=f"sume{b}")
                nc.vector.tensor_add(sume, se[0], se[1])
                rec = tinyp.tile([1, 1], F32, tag="t1", name=f"rec{b}")
                nc.vector.reciprocal(rec, sume)
                nc.scalar.activation(
                    out=gs["expn4"][32 * i : 32 * i + 1, :], in_=exp_row,
                    func=AF.Copy, bias=0.0, scale=rec[:, :1],
                )
                # weight row replicated across partitions via DRAM bounce
                ebounce = dramp.tile([1, V], BF16, tag="eb", name=f"eb{b}")
                nc.sync.dma_start(out=ebounce, in_=gs["expn4"][32 * i : 32 * i + 1, :])
                ebcs = ebsb.tile([128, V], BF16, tag="ebcs", name=f"ebcs{b}")
                nc.sync.dma_start(
                    out=ebcs,
                    in_=bass.AP(
                        tensor=ebounce.tensor, offset=ebounce.offset,
                        ap=[[0, 128]] + ebounce.ap[1:],
                    ),
                )
                st[b]["ebc"] = ebcs

            def emit_ws_a(g, i):
                gs = gst[g]
                b = g * GR + i
                ws2 = tinyp.tile([128, DC], F32, tag="ws2", name=f"ws2_{b}")
                wscr = scrp.tile([128, V], BF16, tag="wscr")
                for c in range(DC):
                    nc.vector.scalar_tensor_tensor(
                        out=wscr, in0=st[b]["te"][:, c, :], scalar=0.0,
                        in1=st[b]["ebc"], op0=ALU.bypass, op1=ALU.mult,
                        accum_out=ws2[:, c : c + 1],
                    )
                st[b]["ws2"] = ws2

            def emit_ws_b(g, i):
                gs = gst[g]
                b = g * GR + i
                if "ws_all" not in gs:
                    gs["ws_all"] = grpp.tile(
                        [128, DC, GR], BF16, tag="wsall", name=f"wsa{g}"
                    )
                ws2 = st[b]["ws2"]
                for c in range(DC):
                    nc.vector.scalar_tensor_tensor(
                        out=gs["ws_all"][:, c, i : i + 1], in0=ws2[:, c : c + 1],
                        scalar=condT[:, c, b : b + 1], in1=zero1,
                        op0=ALU.add, op1=ALU.add,
                    )

            def emit_ws(g, i):
                emit_ws_a(g, i)
                emit_ws_b(g, i)

            def base_chunk_thunks(g, jb):
                """baseT[:, jb, :] = (M3.T @ ws + r0) chunk, as separate ops so
                they can interleave with H chains (hides the m3 LDWEIGHTS)."""
                gs = gst[g]
                if "baseT" not in gs:
                    gs["baseT"] = grpp.tile(
                        [128, JB, GR], F32, tag="baseT", name=f"bT{g}"
                    )
                box = {}

                def t_fold():
                    box["btp"] = hp.tile(
                        [128, GR], F32, tag="h", name=f"btp{g}_{jb}"
                    )
                    nc.tensor.matmul(
                        box["btp"], r01[0:1, jb * 128 : (jb + 1) * 128],
                        ones_bf[0:1, 0:GR], start=True, stop=False,
                    )

                def t_mm(c):
                    nc.tensor.matmul(
                        box["btp"], m3[:, c, jb * 128 : (jb + 1) * 128],
                        gs["ws_all"][:, c, :],
                        start=False, stop=(c == DC - 1),
                    )

                def t_copy():
                    nc.scalar.copy(out=gs["baseT"][:, jb, :], in_=box["btp"])

                return [t_fold] + [
                    (lambda c0=c: t_mm(c0)) for c in range(DC)
                ] + [t_copy]

            def emit_base_chunk(g, jb):
                for t in base_chunk_thunks(g, jb):
                    t()

            def emit_decode(g, sprinkle, post_reduce=None):
                gs = gst[g]
                p4 = pp.tile([128, 2, 512], F32, tag="p4", name=f"p4_{g}")
                inited = [False]

                def emit_init():
                    # p = w (softmax weights) per packed row; deferred so it
                    # doesn't stall PE on the previous group's epilogue reads
                    for vh in range(2):
                        nc.tensor.matmul(
                            p4[:, vh, :], sel4,
                            gs["expn4"][:, vh * 512 : (vh + 1) * 512],
                            start=True, stop=False, skip_group_check=True,
                        )
                    inited[0] = True

                pending = None

                def emit_reduce(jb, vh, hrs):
                    if not inited[0]:
                        emit_init()
                    for i in range(GR):
                        nc.tensor.matmul(
                            p4[32 * i : 32 * i + 1, vh, :],
                            w2sb[:, jb : jb + 1], hrs[i],
                            start=False, stop=(jb == JB - 1),
                            skip_group_check=True,
                            tile_position=(0, 32 * i),
                        )

                base_q = []
                for jb2 in range(1, JB):
                    base_q.extend(base_chunk_thunks(g, jb2))

                def pop_base(n):
                    while n > 0 and base_q:
                        base_q.pop(0)()
                        n -= 1

                for jb in range(JB):
                    for vh in range(2):
                        hrs = []
                        for i in range(GR):
                            b = g * GR + i
                            hps = hp.tile(
                                [128, 512], F32, tag="h", name=f"h{b}_{jb}_{vh}"
                            )
                            for c in range(DC):
                                nc.tensor.matmul(
                                    hps,
                                    bm[:, c, jb * 128 : (jb + 1) * 128],
                                    st[b]["te"][:, c, vh * 512 : (vh + 1) * 512],
                                    start=(c == 0), stop=(c == DC - 1),
                                )
                            hr = hrp.tile(
                                [128, 512], BF16, tag="hr", name=f"hr{b}_{jb}_{vh}"
                            )
                            nc.scalar.activation(
                                out=hr, in_=hps, func=AF.Relu,
                                bias=gs["baseT"][:, jb, i : i + 1], scale=1.0,
                            )
                            hrs.append(hr)
                            pop_base(2)
                        if NO_DELAY:
                            emit_reduce(jb, vh, hrs)
                        else:
                            if pending is not None:
                                emit_reduce(*pending)
                            pending = (jb, vh, hrs)
                    for fn in sprinkle.get(jb, []):
                        fn()
                while base_q:
                    base_q.pop(0)()
                if pending is not None:
                    emit_reduce(*pending)
                if post_reduce is not None:
                    post_reduce()
                # epilogue: p rows -> SBUF (+bd2) -> DRAM
                for i in range(GR):
                    b = g * GR + i
                    p_row = rowp.tile([1, 2, 512], F32, tag="prow", name=f"prow{b}")
                    nc.scalar.activation(
                        out=p_row, in_=p4[32 * i : 32 * i + 1, :, :],
                        func=AF.Identity, bias=bd2_val, scale=1.0,
                    )
                    nc.sync.dma_start(
                        out=p_d[b : b + 1, :],
                        in_=p_row.rearrange("p a v -> p (a v)"),
                    )
                    st[b].clear()

            # ================= pipeline =================
            emit_x4_load(0)
            for i in range(GR):
                emit_te_load(0, i)
            nc.sync.dma_start(out=m3, in_=m3_d[:].rearrange("(c p) j -> p c j", p=128))
            nc.sync.dma_start(out=bm, in_=bm_d[:].rearrange("(c p) j -> p c j", p=128))
            emit_x4_load(1)
            for i in range(GR):
                emit_te_load(1, i)
            emit_scores(0)
            for i in range(GR):
                emit_softmax(0, i)
                emit_ws(0, i)
            emit_base_chunk(0, 0)
            for g in range(NG):
                spr = {jb: [] for jb in range(JB)}
                # next-group prep (optionally deferred for debugging)
                nxt = {jb: [] for jb in range(JB)}
                deferred = []
                lvl = 0 if NO_SPRINKLE else SPR_LEVEL

                def _put(level, jb, fn):
                    if lvl >= level:
                        nxt[jb].append(fn)
                    else:
                        deferred.append(fn)

                if g + 2 < NG:
                    _put(1, 0, lambda g2=g + 2: emit_x4_load(g2))
                    for i in range(GR):
                        _put(1, 1 + i, lambda g2=g + 2, i2=i: emit_te_load(g2, i2))
                if g + 1 < NG:
                    _put(2, 0, lambda g1=g + 1: emit_scores(g1))
                    for i in range(GR):
                        _put(3, 1 + i, lambda g1=g + 1, i2=i: emit_softmax(g1, i2))
                        _put(4, 2 + i, lambda g1=g + 1, i2=i: emit_ws_a(g1, i2))
                        _put(5, 3 + i, lambda g1=g + 1, i2=i: emit_ws_b(g1, i2))
                flat = deferred
                for jb in range(JB):
                    spr[jb].extend(nxt[jb])
                post = (
                    (lambda g1=g + 1: emit_base_chunk(g1, 0))
                    if g + 1 < NG else None
                )
                if lvl >= 5:
                    emit_decode(g, spr, post_reduce=post)
                    for fn in flat:
                        fn()
                else:
                    emit_decode(g, spr)
                    for fn in flat:
                        fn()
                    if post is not None:
                        post()

    return nc


_NC_CACHE: dict = {}


def _get_nc(bd2_val: float) -> bass.Bass:
    key = float(bd2_val)
    if key not in _NC_CACHE:
        nc = build_nc(key)
        nc.finalize()
        _NC_CACHE[key] = nc
    return _NC_CACHE[key]


def _pos_encoding() -> np.ndarray:
    pos = np.arange(MAX_LEN, dtype=np.float32)[:, None]
    div = np.exp(np.arange(0, D, 2, dtype=np.float32) * (-np.log(10000.0) / D))
    pe = np.zeros((MAX_LEN, D), dtype=np.float32)
    pe[:, 0::2] = np.sin(pos * div)
    pe[:, 1::2] = np.cos(pos * div)
    return pe


def prepare_in_maps(inputs: dict) -> tuple[list, float]:
    f32 = lambda a: np.ascontiguousarray(np.asarray(a), dtype=np.float32)
    bf = lambda a: np.ascontiguousarray(np.asarray(a, dtype=np.float32).astype(BF_NP))
    x = np.asarray(inputs["x"], dtype=np.float32)
    ts = np.ascontiguousarray(np.asarray(inputs["timesteps"]).astype(np.int32).reshape(B, 1))
    qe = np.asarray(inputs["query_emb"], dtype=np.float32)
    te = np.asarray(inputs["target_emb"], dtype=np.float32)
    Wq, Wk, Wv, Wp = (f32(inputs[k]) for k in ("Wq", "Wk", "Wv", "Wp"))
    bp = f32(inputs["bp"])
    Wt1, bt1, Wt2, bt2 = (f32(inputs[k]) for k in ("Wt1", "bt1", "Wt2", "bt2"))
    Wd1, bd1, Wd2, bd2 = (f32(inputs[k]) for k in ("Wd1", "bd1", "Wd2", "bd2"))

    pe = _pos_encoding()
    M1 = Wq.T @ Wk
    A = np.ascontiguousarray(Wd1[:, :D].T)
    Bm = Wd1[:, D:].T
    M3 = (Wv.T @ Wp.T) @ A
    r0 = np.ascontiguousarray(bp @ A + bd1)
    w2 = np.ascontiguousarray(Wd2[0])
    bd2_val = float(bd2.reshape(-1)[0])
    bt1c = np.ascontiguousarray(bt1.reshape(DC, 128).T)
    bt2c = np.ascontiguousarray(bt2.reshape(DC, 128).T)
    w2c = np.ascontiguousarray(w2.reshape(JB, 128).T)
    sel4 = np.zeros((128, 128), dtype=np.float32)
    for i in range(GR):
        sel4[32 * i, 32 * i] = 1.0

    shared = dict(
        pe=pe, wt1t=bf(Wt1.T), wt2t=bf(Wt2.T), bt1c=bt1c, bt2c=bt2c,
        m1=bf(M1), m3=bf(M3), bm=bf(Bm), r0=bf(r0), w2c=bf(w2c), sel4=bf(sel4),
    )
    in_maps = []
    for i in range(NCORES):
        s = slice(i * BSH, (i + 1) * BSH)
        in_maps.append(
            dict(
                te=bf(te[s].transpose(0, 2, 1)),
                x=bf(x[s]),
                ts=np.ascontiguousarray(ts[s]),
                qet=bf(qe[s].T),
                **shared,
            )
        )
    return in_maps, bd2_val


def run(inputs: dict, trace: bool = False):
    in_maps, bd2_val = prepare_in_maps(inputs)
    nc = _get_nc(bd2_val)
    res = run_bass_kernel_spmd(nc, in_maps, list(range(NCORES)), trace=trace)
    out = np.concatenate([r["p"] for r in res.results], axis=0).astype(np.float32)
    return out, res


def kernel(**inputs) -> np.ndarray:
    out, _ = run(inputs, trace=False)
    return out


# revision 27
# speedup vs baseline: 1.2085x; 1.2085x over previous
"""Trainium2 Bass kernel for the single-query-attention diffusion decoder.

Full-input contract: kernel(**inputs) -> np.ndarray [B, V].
Data-parallel over batch across 8 NeuronCores (16 rows each).

Math (reference restructured):
    cond  = silu(pe[t] @ Wt1.T + bt1) @ Wt2.T + bt2            [B, D]
    q~    = (query + cond) @ M1,  M1 = Wq.T @ Wk               [B, D]
    s[v]  = q~ . T[v] + x[v]   (softmax shift-invariant)
    w     = softmax(s)
    ws    = sum_v w[v] T[v] + cond                             [D]
    base  = ws @ M3 + r0                                       [J]
            M3 = Wv.T @ Wp.T @ Wd1[:, :D].T,  r0 = bp @ Wd1[:, :D].T + bd1
    p[v]  = sum_j relu(T[v] @ Bm + base)[j] * w2[j] + bd2 + w[v]
            Bm = Wd1[:, D:].T

v2 layout: the decoder H matmul runs in [j, v] orientation (lhsT = Bm
chunk stationary, te streams) so the per-j base fold is a free ACT-bias
in the relu, and the w2 reduction over j is a K=128 matvec.  Rows are
processed in groups of 4; the M=1 matvecs (scores, w2-reduce) for the 4
rows are column-packed into one PSUM bank at partitions 0/32/64/96 and
emitted back-to-back so they overlap in the PE array.  Shared banks are
initialized by a single full-partition K=4 "selector" matmul that also
folds the additive row term (x for scores, softmax weights for p), and
all per-row chains accumulate with start=False.  The base matvec is
batched (ws of 4 rows as N) and emitted chunk-wise between H bursts.
"""

import os
import sys

for _p in ("/opt/trn_rl_repo", "/opt/trn_rl_repo/concourse"):
    if os.path.isdir(_p) and _p not in sys.path:
        sys.path.append(_p)

import numpy as np
import ml_dtypes

import concourse.bass as bass
import concourse.tile as tile
from concourse import bacc, mybir
from concourse.bass_utils import run_bass_kernel_spmd

F32 = mybir.dt.float32
BF16 = mybir.dt.bfloat16
I32 = mybir.dt.int32
AF = mybir.ActivationFunctionType
ALU = mybir.AluOpType
BF_NP = ml_dtypes.bfloat16

NCORES = 8
B = 128
BSH = B // NCORES  # 16 batch rows per core
D = 512
V = 1024
J = 2 * D  # 1024 decoder hidden
DC = D // 128  # 4 d-chunks
JB = J // 128  # 8 j-chunks
GR = 2  # rows per group (column-packed)
NG = BSH // GR  # 4 groups
MAX_LEN = 5000
NO_SPRINKLE = bool(int(os.environ.get("K_NO_SPRINKLE", "0")))
NO_DELAY = bool(int(os.environ.get("K_NO_DELAY", "0")))
# 0=defer all next-group prep, 1=sprinkle loads, 2=+scores, 3=+softmax/ws (all)
SPR_LEVEL = int(os.environ.get("K_SPR_LEVEL", "5"))


def build_nc(bd2_val: float) -> bass.Bass:
    nc = bacc.Bacc()

    # ---- per-core inputs ----
    te_d = nc.declare_dram_parameter("te", [BSH, D, V], BF16, isOutput=False)
    x_d = nc.declare_dram_parameter("x", [BSH, V], BF16, isOutput=False)
    ts_d = nc.declare_dram_parameter("ts", [BSH, 1], I32, isOutput=False)
    qet_d = nc.declare_dram_parameter("qet", [D, BSH], BF16, isOutput=False)
    # ---- replicated (host-folded) weights ----
    pe_d = nc.declare_dram_parameter("pe", [MAX_LEN, D], F32, isOutput=False)
    wt1t_d = nc.declare_dram_parameter("wt1t", [D, D], BF16, isOutput=False)
    wt2t_d = nc.declare_dram_parameter("wt2t", [D, D], BF16, isOutput=False)
    bt1c_d = nc.declare_dram_parameter("bt1c", [128, DC], F32, isOutput=False)
    bt2c_d = nc.declare_dram_parameter("bt2c", [128, DC], F32, isOutput=False)
    m1_d = nc.declare_dram_parameter("m1", [D, D], BF16, isOutput=False)
    m3_d = nc.declare_dram_parameter("m3", [D, J], BF16, isOutput=False)
    bm_d = nc.declare_dram_parameter("bm", [D, J], BF16, isOutput=False)
    r0c_d = nc.declare_dram_parameter("r0c", [128, JB], F32, isOutput=False)
    w2c_d = nc.declare_dram_parameter("w2c", [128, JB], BF16, isOutput=False)
    sel4_d = nc.declare_dram_parameter("sel4", [128, 128], BF16, isOutput=False)
    p_d = nc.declare_dram_parameter("p", [BSH, V], F32, isOutput=True)

    with tile.TileContext(nc) as tc:
        with (
            tc.tile_pool(name="w", bufs=1) as wp,
            tc.tile_pool(name="te", bufs=12) as tep,
            tc.tile_pool(name="scr", bufs=1) as scrp,
            tc.tile_pool(name="rows", bufs=2) as rowp,
            tc.tile_pool(name="hr", bufs=8) as hrp,
            tc.tile_pool(name="tiny", bufs=6) as tinyp,
            tc.tile_pool(name="grp", bufs=2) as grpp,
            tc.tile_pool(name="ebsb", bufs=2) as ebsb,
            tc.tile_pool(name="dramp", bufs=2, space="DRAM") as dramp,
            tc.tile_pool(name="hp", bufs=4, space="PSUM") as hp,  # 4 banks
            tc.tile_pool(name="pp", bufs=1, space="PSUM") as pp,  # 2 banks
            tc.tile_pool(name="scp", bufs=2, space="PSUM") as scp,  # 2 banks
        ):
            # ================= weight / constant loads =================
            # setup-critical small tensors first so the cond/q~ chain and the
            # first group's scores aren't queued behind bulk weights
            ts_sb = wp.tile([BSH, 1], I32, tag="ts")
            nc.sync.dma_start(out=ts_sb, in_=ts_d[:])
            bt1c = wp.tile([128, DC], F32, tag="bt1c")
            nc.sync.dma_start(out=bt1c, in_=bt1c_d[:])
            bt2c = wp.tile([128, DC], F32, tag="bt2c")
            nc.sync.dma_start(out=bt2c, in_=bt2c_d[:])
            qet = wp.tile([128, DC, BSH], BF16, tag="qet")
            nc.sync.dma_start(out=qet, in_=qet_d[:].rearrange("(c p) b -> p c b", p=128))
            wt1t = wp.tile([128, DC, D], BF16, tag="wt1t")
            nc.sync.dma_start(out=wt1t, in_=wt1t_d[:].rearrange("(c p) z -> p c z", p=128))
            wt2t = wp.tile([128, DC, D], BF16, tag="wt2t")
            nc.sync.dma_start(out=wt2t, in_=wt2t_d[:].rearrange("(c p) z -> p c z", p=128))
            m1 = wp.tile([128, DC, D], BF16, tag="m1")
            nc.sync.dma_start(out=m1, in_=m1_d[:].rearrange("(c p) z -> p c z", p=128))
            m3 = wp.tile([128, DC, J], BF16, tag="m3")
            bm = wp.tile([128, DC, J], BF16, tag="bm")
            # r0 chunked [p, jb]; per-partition bias in the baseT copy
            r0c = wp.tile([128, JB], F32, tag="r0c")
            nc.sync.dma_start(out=r0c, in_=r0c_d[:])
            # w2 chunked [p, jb] (j = jb*128 + p); lhsT columns for the reduce
            w2sb = wp.tile([128, JB], BF16, tag="w2sb")
            nc.sync.dma_start(out=w2sb, in_=w2c_d[:])
            ones_bf = wp.tile([1, 128], BF16, tag="ones_bf")
            nc.vector.memset(ones_bf, 1.0)
            # selector: sel4[32k, 32k] = 1, else 0 -- bank-init matmuls
            sel4 = wp.tile([128, 128], BF16, tag="sel4")
            nc.sync.dma_start(out=sel4, in_=sel4_d[:])
            id128 = wp.tile([128, 128], F32, tag="id128")
            from concourse.masks import make_identity

            make_identity(nc, id128)
            id_bf = wp.tile([BSH, BSH], BF16, tag="id_bf")
            nc.scalar.activation(out=id_bf, in_=id128[:BSH, :BSH], func=AF.Copy)
            # PE warmup so later transposes never owe a Pool wait
            warm_ps = hp.tile([2, 2], F32, tag="h")
            nc.tensor.transpose(warm_ps, id128[0:2, 0:2], id128[0:2, 0:2])
            zero1 = wp.tile([128, 1], F32, tag="zero1")
            nc.vector.memset(zero1, 0.0)
            zerosV = wp.tile([128, V], BF16, tag="zerosV")
            nc.vector.memset(zerosV, 0.0)

            # ================= setup: cond / q~ =================
            tpe = wp.tile([BSH, D], F32, tag="tpe")
            nc.gpsimd.indirect_dma_start(
                out=tpe[:],
                out_offset=None,
                in_=pe_d[:],
                in_offset=bass.IndirectOffsetOnAxis(ap=ts_sb[:, :1], axis=0),
            )
            tpe_bf = wp.tile([BSH, D], BF16, tag="tpe_bf")
            nc.scalar.activation(out=tpe_bf, in_=tpe, func=AF.Copy)
            tpeT = wp.tile([128, DC, BSH], BF16, tag="tpeT")
            for c in range(DC):
                ps = hp.tile([128, BSH], BF16, tag="h", name=f"tps{c}")
                nc.tensor.transpose(ps, tpe_bf[:, c * 128 : (c + 1) * 128], id_bf)
                nc.scalar.activation(out=tpeT[:, c, :], in_=ps, func=AF.Copy)
            # Z.T = Wt1 @ tpe.T (+bt1), silu
            s_sb = wp.tile([128, DC, BSH], BF16, tag="s_sb")
            for zt in range(DC):
                ps = hp.tile([128, BSH], F32, tag="h", name=f"zps{zt}")
                for c in range(DC):
                    nc.tensor.matmul(
                        ps, wt1t[:, c, zt * 128 : (zt + 1) * 128], tpeT[:, c, :],
                        start=(c == 0), stop=(c == DC - 1),
                    )
                # silu(z) = z * sigmoid(z); sim has no Silu
                zt_sb = wp.tile([128, DC, BSH], BF16, tag="zt_sb")
                nc.scalar.activation(
                    out=zt_sb[:, zt, :], in_=ps, func=AF.Identity,
                    bias=bt1c[:, zt : zt + 1], scale=1.0,
                )
                sg_sb = wp.tile([128, DC, BSH], BF16, tag="sg_sb")
                nc.scalar.activation(
                    out=sg_sb[:, zt, :], in_=ps, func=AF.Sigmoid,
                    bias=bt1c[:, zt : zt + 1], scale=1.0,
                )
                nc.vector.tensor_mul(
                    s_sb[:, zt, :], zt_sb[:, zt, :], sg_sb[:, zt, :]
                )
            # condT = Wt2 @ silu (+bt2)
            condT = wp.tile([128, DC, BSH], BF16, tag="condT")
            for ct in range(DC):
                ps = hp.tile([128, BSH], F32, tag="h", name=f"cps{ct}")
                for c in range(DC):
                    nc.tensor.matmul(
                        ps, wt2t[:, c, ct * 128 : (ct + 1) * 128], s_sb[:, c, :],
                        start=(c == 0), stop=(c == DC - 1),
                    )
                nc.scalar.activation(
                    out=condT[:, ct, :], in_=ps, func=AF.Identity,
                    bias=bt2c[:, ct : ct + 1], scale=1.0,
                )
            # qcT = qeT + condT ; q~T = M1.T @ qcT
            qcT = wp.tile([128, DC, BSH], BF16, tag="qcT")
            nc.vector.tensor_add(qcT[:], qet[:], condT[:])
            qtT = wp.tile([128, DC, BSH], BF16, tag="qtT")
            for mt in range(DC):
                ps = hp.tile([128, BSH], F32, tag="h", name=f"qps{mt}")
                for c in range(DC):
                    nc.tensor.matmul(
                        ps, m1[:, c, mt * 128 : (mt + 1) * 128], qcT[:, c, :],
                        start=(c == 0), stop=(c == DC - 1),
                    )
                nc.scalar.activation(out=qtT[:, mt, :], in_=ps, func=AF.Copy)

            # ================= per-row / per-group state =================
            st = [dict() for _ in range(BSH)]
            gst = [dict() for _ in range(NG)]

            def emit_x4_load(g):
                gs = gst[g]
                x4 = grpp.tile([128, V], BF16, tag="x4", name=f"x4_{g}")
                nc.vector.memset(x4, 0.0)
                for i in range(GR):
                    b = g * GR + i
                    nc.sync.dma_start(
                        out=x4[32 * i : 32 * i + 1, :], in_=x_d[b : b + 1, :]
                    )
                gs["x4"] = x4

            def emit_te_load(g, i):
                b = g * GR + i
                te_t = tep.tile([128, DC, V], BF16, tag="te", name=f"te{b}")
                for c in range(DC):
                    nc.sync.dma_start(
                        out=te_t[:, c, :], in_=te_d[b, c * 128 : (c + 1) * 128, :]
                    )
                st[b]["te"] = te_t

            def emit_scores(g):
                """sc[j32i, v] for 4 rows column-packed; x folded via bank init."""
                gs = gst[g]
                sc = []
                for h in range(2):
                    scps = scp.tile([128, 512], F32, tag="sc", name=f"sc{g}_{h}")
                    nc.tensor.matmul(
                        scps, sel4, gs["x4"][:, h * 512 : (h + 1) * 512],
                        start=True, stop=False, skip_group_check=True,
                    )
                    for c in range(DC):
                        for i in range(GR):
                            b = g * GR + i
                            nc.tensor.matmul(
                                scps[32 * i : 32 * i + 1, :],
                                qtT[:, c, b : b + 1],
                                st[b]["te"][:, c, h * 512 : (h + 1) * 512],
                                start=False, stop=(c == DC - 1),
                                skip_group_check=True,
                                tile_position=(0, 32 * i),
                            )
                    sc.append(scps)
                gs["sc"] = sc

            def emit_softmax(g, i):
                gs = gst[g]
                b = g * GR + i
                if "expn4" not in gs:
                    gs["expn4"] = grpp.tile([128, V], BF16, tag="expn4", name=f"en{g}")
                    nc.vector.memset(gs["expn4"], 0.0)
                exp_row = rowp.tile([1, V], F32, tag="exp", name=f"exp{b}")
                se = [
                    tinyp.tile([1, 1], F32, tag="t1", name=f"se{h}_{b}")
                    for h in range(2)
                ]
                for h in range(2):
                    nc.scalar.activation(
                        out=exp_row[:, h * 512 : (h + 1) * 512],
                        in_=gs["sc"][h][32 * i : 32 * i + 1, :],
                        func=AF.Exp, accum_out=se[h],
                    )
                sume = tinyp.tile([1, 1], F32, tag="t1", name=f"sume{b}")
                nc.vector.tensor_add(sume, se[0], se[1])
                rec = tinyp.tile([1, 1], F32, tag="t1", name=f"rec{b}")
                nc.vector.reciprocal(rec, sume)
                nc.scalar.activation(
                    out=gs["expn4"][32 * i : 32 * i + 1, :], in_=exp_row,
                    func=AF.Copy, bias=0.0, scale=rec[:, :1],
                )
                # weight row replicated across partitions via DRAM bounce
                ebounce = dramp.tile([1, V], BF16, tag="eb", name=f"eb{b}")
                nc.sync.dma_start(out=ebounce, in_=gs["expn4"][32 * i : 32 * i + 1, :])
                ebcs = ebsb.tile([128, V], BF16, tag="ebcs", name=f"ebcs{b}")
                nc.sync.dma_start(
                    out=ebcs,
                    in_=bass.AP(
                        tensor=ebounce.tensor, offset=ebounce.offset,
                        ap=[[0, 128]] + ebounce.ap[1:],
                    ),
                )
                st[b]["ebc"] = ebcs

            def emit_ws_a(g, i):
                gs = gst[g]
                b = g * GR + i
                ws2 = tinyp.tile([128, DC], F32, tag="ws2", name=f"ws2_{b}")
                wscr = scrp.tile([128, V], BF16, tag="wscr")
                for c in range(DC):
                    nc.vector.scalar_tensor_tensor(
                        out=wscr, in0=st[b]["te"][:, c, :], scalar=0.0,
                        in1=st[b]["ebc"], op0=ALU.bypass, op1=ALU.mult,
                        accum_out=ws2[:, c : c + 1],
                    )
                st[b]["ws2"] = ws2

            def emit_ws_b(g, i):
                gs = gst[g]
                b = g * GR + i
                if "ws_all" not in gs:
                    gs["ws_all"] = grpp.tile(
                        [128, DC, GR], BF16, tag="wsall", name=f"wsa{g}"
                    )
                ws2 = st[b]["ws2"]
                for c in range(DC):
                    nc.vector.scalar_tensor_tensor(
                        out=gs["ws_all"][:, c, i : i + 1], in0=ws2[:, c : c + 1],
                        scalar=condT[:, c, b : b + 1], in1=zero1,
                        op0=ALU.add, op1=ALU.add,
                    )

            def emit_ws(g, i):
                emit_ws_a(g, i)
                emit_ws_b(g, i)

            def base_chunk_thunks(g, jb):
                """baseT[:, jb, :] = (M3.T @ ws + r0) chunk, as separate ops so
                they can interleave with H chains (hides the m3 LDWEIGHTS)."""
                gs = gst[g]
                if "baseT" not in gs:
                    gs["baseT"] = grpp.tile(
                        [128, JB, GR], F32, tag="baseT", name=f"bT{g}"
                    )
                box = {}

                def t_mm(c):
                    if c == 0:
                        box["btp"] = hp.tile(
                            [128, GR], F32, tag="h", name=f"btp{g}_{jb}"
                        )
                    nc.tensor.matmul(
                        box["btp"], m3[:, c, jb * 128 : (jb + 1) * 128],
                        gs["ws_all"][:, c, :],
                        start=(c == 0), stop=(c == DC - 1),
                    )

                def t_copy():
                    nc.scalar.activation(
                        out=gs["baseT"][:, jb, :], in_=box["btp"],
                        func=AF.Identity, bias=r0c[:, jb : jb + 1], scale=1.0,
                    )

                return [
                    (lambda c0=c: t_mm(c0)) for c in range(DC)
                ] + [t_copy]

            def emit_base_chunk(g, jb):
                for t in base_chunk_thunks(g, jb):
                    t()

            def emit_decode(g, sprinkle, post_reduce=None):
                gs = gst[g]
                p4 = pp.tile([128, 2, 512], F32, tag="p4", name=f"p4_{g}")
                inited = [False]

                def emit_init():
                    # p = w (softmax weights) per packed row; deferred so it
                    # doesn't stall PE on the previous group's epilogue reads
                    for vh in range(2):
                        nc.tensor.matmul(
                            p4[:, vh, :], sel4,
                            gs["expn4"][:, vh * 512 : (vh + 1) * 512],
                            start=True, stop=False, skip_group_check=True,
                        )
                    inited[0] = True

                pending = None

                def emit_reduce(jb, vh, hrs):
                    if not inited[0]:
                        emit_init()
                    for i in range(GR):
                        nc.tensor.matmul(
                            p4[32 * i : 32 * i + 1, vh, :],
                            w2sb[:, jb : jb + 1], hrs[i],
                            start=False, stop=(jb == JB - 1),
                            skip_group_check=True,
                            tile_position=(0, 32 * i),
                        )

                base_q = []
                for jb2 in range(1, JB):
                    base_q.extend(base_chunk_thunks(g, jb2))

                def pop_base(n):
                    while n > 0 and base_q:
                        base_q.pop(0)()
                        n -= 1

                for jb in range(JB):
                    for vh in range(2):
                        hrs = []
                        for i in range(GR):
                            b = g * GR + i
                            hps = hp.tile(
                                [128, 512], F32, tag="h", name=f"h{b}_{jb}_{vh}"
                            )
                            for c in range(DC):
                                nc.tensor.matmul(
                                    hps,
                                    bm[:, c, jb * 128 : (jb + 1) * 128],
                                    st[b]["te"][:, c, vh * 512 : (vh + 1) * 512],
                                    start=(c == 0), stop=(c == DC - 1),
                                )
                            hr = hrp.tile(
                                [128, 512], BF16, tag="hr", name=f"hr{b}_{jb}_{vh}"
                            )
                            nc.scalar.activation(
                                out=hr, in_=hps, func=AF.Relu,
                                bias=gs["baseT"][:, jb, i : i + 1], scale=1.0,
                            )
                            hrs.append(hr)
                            pop_base(2)
                        if NO_DELAY:
                            emit_reduce(jb, vh, hrs)
                        else:
                            if pending is not None:
                                emit_reduce(*pending)
                            pending = (jb, vh, hrs)
                    for fn in sprinkle.get(jb, []):
                        fn()
                while base_q:
                    base_q.pop(0)()
                if pending is not None:
                    emit_reduce(*pending)
                if post_reduce is not None:
                    post_reduce()
                # epilogue: p rows -> SBUF (+bd2) -> DRAM
                for i in range(GR):
                    b = g * GR + i
                    p_row = rowp.tile([1, 2, 512], F32, tag="prow", name=f"prow{b}")
                    nc.scalar.activation(
                        out=p_row, in_=p4[32 * i : 32 * i + 1, :, :],
                        func=AF.Identity, bias=bd2_val, scale=1.0,
                    )
                    nc.sync.dma_start(
                        out=p_d[b : b + 1, :],
                        in_=p_row.rearrange("p a v -> p (a v)"),
                    )
                    st[b].clear()

            # ================= pipeline =================
            emit_x4_load(0)
            for i in range(GR):
                emit_te_load(0, i)
            nc.sync.dma_start(out=m3, in_=m3_d[:].rearrange("(c p) j -> p c j", p=128))
            nc.sync.dma_start(out=bm, in_=bm_d[:].rearrange("(c p) j -> p c j", p=128))
            emit_x4_load(1)
            for i in range(GR):
                emit_te_load(1, i)
            emit_scores(0)
            for i in range(GR):
                emit_softmax(0, i)
                emit_ws(0, i)
            emit_base_chunk(0, 0)
            for g in range(NG):
                spr = {jb: [] for jb in range(JB)}
                # next-group prep (optionally deferred for debugging)
                nxt = {jb: [] for jb in range(JB)}
                deferred = []
                lvl = 0 if NO_SPRINKLE else SPR_LEVEL

                def _put(level, jb, fn):
                    if lvl >= level:
                        nxt[jb].append(fn)
                    else:
                        deferred.append(fn)

                if g + 2 < NG:
                    _put(1, 0, lambda g2=g + 2: emit_x4_load(g2))
                    for i in range(GR):
                        _put(1, 1 + i, lambda g2=g + 2, i2=i: emit_te_load(g2, i2))
                if g + 1 < NG:
                    _put(2, 0, lambda g1=g + 1: emit_scores(g1))
                    for i in range(GR):
                        _put(3, 1 + i, lambda g1=g + 1, i2=i: emit_softmax(g1, i2))
                        _put(4, 2 + i, lambda g1=g + 1, i2=i: emit_ws_a(g1, i2))
                        _put(5, 3 + i, lambda g1=g + 1, i2=i: emit_ws_b(g1, i2))
                flat = deferred
                for jb in range(JB):
                    spr[jb].extend(nxt[jb])
                post = (
                    (lambda g1=g + 1: emit_base_chunk(g1, 0))
                    if g + 1 < NG else None
                )
                if lvl >= 5:
                    emit_decode(g, spr, post_reduce=post)
                    for fn in flat:
                        fn()
                else:
                    emit_decode(g, spr)
                    for fn in flat:
                        fn()
                    if post is not None:
                        post()

    return nc


_NC_CACHE: dict = {}


def _get_nc(bd2_val: float) -> bass.Bass:
    key = float(bd2_val)
    if key not in _NC_CACHE:
        nc = build_nc(key)
        nc.finalize()
        _NC_CACHE[key] = nc
    return _NC_CACHE[key]


def _pos_encoding() -> np.ndarray:
    pos = np.arange(MAX_LEN, dtype=np.float32)[:, None]
    div = np.exp(np.arange(0, D, 2, dtype=np.float32) * (-np.log(10000.0) / D))
    pe = np.zeros((MAX_LEN, D), dtype=np.float32)
    pe[:, 0::2] = np.sin(pos * div)
    pe[:, 1::2] = np.cos(pos * div)
    return pe


def prepare_in_maps(inputs: dict) -> tuple[list, float]:
    f32 = lambda a: np.ascontiguousarray(np.asarray(a), dtype=np.float32)
    bf = lambda a: np.ascontiguousarray(np.asarray(a, dtype=np.float32).astype(BF_NP))
    x = np.asarray(inputs["x"], dtype=np.float32)
    ts = np.ascontiguousarray(np.asarray(inputs["timesteps"]).astype(np.int32).reshape(B, 1))
    qe = np.asarray(inputs["query_emb"], dtype=np.float32)
    te = np.asarray(inputs["target_emb"], dtype=np.float32)
    Wq, Wk, Wv, Wp = (f32(inputs[k]) for k in ("Wq", "Wk", "Wv", "Wp"))
    bp = f32(inputs["bp"])
    Wt1, bt1, Wt2, bt2 = (f32(inputs[k]) for k in ("Wt1", "bt1", "Wt2", "bt2"))
    Wd1, bd1, Wd2, bd2 = (f32(inputs[k]) for k in ("Wd1", "bd1", "Wd2", "bd2"))

    pe = _pos_encoding()
    M1 = Wq.T @ Wk
    A = np.ascontiguousarray(Wd1[:, :D].T)
    Bm = Wd1[:, D:].T
    M3 = (Wv.T @ Wp.T) @ A
    r0 = np.ascontiguousarray(bp @ A + bd1)
    r0c = np.ascontiguousarray(r0.reshape(JB, 128).T.astype(np.float32))
    w2 = np.ascontiguousarray(Wd2[0])
    bd2_val = float(bd2.reshape(-1)[0])
    bt1c = np.ascontiguousarray(bt1.reshape(DC, 128).T)
    bt2c = np.ascontiguousarray(bt2.reshape(DC, 128).T)
    w2c = np.ascontiguousarray(w2.reshape(JB, 128).T)
    sel4 = np.zeros((128, 128), dtype=np.float32)
    for i in range(GR):
        sel4[32 * i, 32 * i] = 1.0

    shared = dict(
        pe=pe, wt1t=bf(Wt1.T), wt2t=bf(Wt2.T), bt1c=bt1c, bt2c=bt2c,
        m1=bf(M1), m3=bf(M3), bm=bf(Bm), r0c=r0c, w2c=bf(w2c), sel4=bf(sel4),
    )
    in_maps = []
    for i in range(NCORES):
        s = slice(i * BSH, (i + 1) * BSH)
        in_maps.append(
            dict(
                te=bf(te[s].transpose(0, 2, 1)),
                x=bf(x[s]),
                ts=np.ascontiguousarray(ts[s]),
                qet=bf(qe[s].T),
                **shared,
            )
        )
    return in_maps, bd2_val


def run(inputs: dict, trace: bool = False):
    in_maps, bd2_val = prepare_in_maps(inputs)
    nc = _get_nc(bd2_val)
    res = run_bass_kernel_spmd(nc, in_maps, list(range(NCORES)), trace=trace)
    out = np.concatenate([r["p"] for r in res.results], axis=0).astype(np.float32)
    return out, res


def kernel(**inputs) -> np.ndarray:
    out, _ = run(inputs, trace=False)
    return out
